# revision 1
# baseline (speedup 1.0000x reference)
"""Multi-head self-attention (B=4, S=2048, D=1024, H=16, causal) on 8 TRN2
NeuronCores.

Sharding: batch x head-group. Core c handles batch b = c//2 and head-group
g = c%2 (8 heads = 512 of the 1024 q/k/v dims). Each core computes a partial
output [S, D] (its head-group's contribution through w_o); the host sums the
two partials per batch and adds b_o.

Per-core kernel (all matmuls fp32r = TF32-like, fp32 accumulate):
  for each q-block (512 queries):
    - project Q (this block) and K/V (this block's keys) from x
    - flash-style attention over k-tiles <= q-block end:
        S^T[k,q] = Kt.T @ Qt  (two heads packed in the PE array via
        tile_position row groups), merged exp on ACT with fused 1/sqrt(dk)
        scale (no max subtraction: |scores| <~ 6 so exp is safe), causal mask
        via multiply on the diagonal tiles, O'^T += V'.T @ P^T where V' has a
        ones column appended so the softmax denominator accumulates for free
    - normalize O^T by the denominator (reciprocal + partition broadcast)
    - output projection of this q-block, DMA partial out
"""

import numpy as np

import concourse.bass as bass
import concourse.mybir as mybir
from concourse import bacc
from concourse.tile import TileContext
from concourse.bass_utils import run_bass_kernel_spmd

B, S, D, H = 4, 2048, 1024, 16
DK = D // H          # 64
N_CORES = 8
GD = D // 2          # 512 dims per head-group
SCALE = 1.0 / float(np.sqrt(DK))

F32 = mybir.dt.float32
F32R = mybir.dt.float32r
EXP = mybir.ActivationFunctionType.Exp

_cache = {}


def _build():
    if "nc" in _cache:
        return _cache["nc"]

    nc = bacc.Bacc("TRN2", target_bir_lowering=False, debug=False,
                   num_devices=N_CORES)

    xT = nc.dram_tensor("xT", (D, S), F32R, kind="ExternalInput")
    wq_t = nc.dram_tensor("wq_t", (D, GD), F32R, kind="ExternalInput")
    wk_t = nc.dram_tensor("wk_t", (D, GD), F32R, kind="ExternalInput")
    wv_t = nc.dram_tensor("wv_t", (D, GD), F32R, kind="ExternalInput")
    wo_t = nc.dram_tensor("wo_t", (GD, D), F32R, kind="ExternalInput")
    masks = nc.dram_tensor("masks", (2, 128, 1024), F32, kind="ExternalInput")
    out_p = nc.dram_tensor("out_p", (S, D), F32, kind="ExternalOutput")

    xT_r = xT.rearrange("(t p) s -> p t s", p=128)        # [128, 8, 2048]
    wq_r = wq_t.rearrange("(t p) d -> p t d", p=128)      # [128, 8, 512]
    wk_r = wk_t.rearrange("(t p) d -> p t d", p=128)
    wv_r = wv_t.rearrange("(t p) d -> p t d", p=128)
    wo_r = wo_t.rearrange("(t p) d -> p t d", p=128)      # [128, 4, 1024]

    with TileContext(nc) as tc:
        with (
            tc.tile_pool(name="pers", bufs=1) as pers,
            tc.tile_pool(name="wp", bufs=1) as wp,
            tc.tile_pool(name="xq", bufs=2) as xq,
            tc.tile_pool(name="wkp", bufs=2) as wkp,
            tc.tile_pool(name="ps", bufs=2, space="PSUM") as ps,
        ):
            # persistent K^T (d-major) and V' (s-major, 65 cols/head)
            kt = [pers.tile([128, S], F32R, name=f"kt{t}") for t in range(4)]
            vp = [pers.tile([128, 8 * (DK + 1)], F32R, name=f"vp{i}")
                  for i in range(16)]

            wq_sb = wp.tile([128, 8, GD], F32R)
            wk_sb = wp.tile([128, 8, GD], F32R)
            wv_sb = wp.tile([128, 8, GD], F32R)
            wo_sb = wp.tile([128, 4, D], F32R)
            mask_sb = wp.tile([128, 2, 1024], F32)
            ones_c = wp.tile([128, 1], F32)
            nc.sync.dma_start(out=wq_sb, in_=wq_r)
            nc.sync.dma_start(out=wk_sb, in_=wk_r)
            nc.sync.dma_start(out=wv_sb, in_=wv_r)
            nc.sync.dma_start(out=wo_sb, in_=wo_r)
            nc.sync.dma_start(out=mask_sb, in_=masks.rearrange("i p q -> p i q"))
            nc.vector.memset(ones_c, 1.0)

            for qb in range(4):
                qs = slice(qb * 512, (qb + 1) * 512)

                # ---- x chunk for this q/s block, in two halves ----
                xh = []
                for h in range(2):
                    xt = xq.tile([128, 4, 512], F32R, tag="xh", name=f"xh{qb}_{h}")
                    nc.sync.dma_start(out=xt, in_=xT_r[:, 4 * h:4 * h + 4, qs])
                    xh.append(xt)

                # ---- Q / K projections for this block ----
                qts = []
                for t in range(4):
                    qt_t = xq.tile([128, 512], F32R, tag="qts", bufs=8,
                                   name=f"qts{qb}_{t}")
                    for dst, wsb in ((qt_t, wq_sb), (None, wk_sb)):
                        pst = ps.tile([128, 512], F32, tag="mm512",
                                      name=f"pp{qb}_{t}")
                        for e in range(8):
                            nc.tensor.matmul(
                                pst,
                                wsb[:, e, t * 128:(t + 1) * 128],
                                xh[e // 4][:, e % 4, :],
                                start=(e == 0), stop=(e == 7),
                            )
                        if dst is None:
                            nc.vector.tensor_copy(kt[t][:, qs], pst)
                        else:
                            nc.vector.tensor_copy(dst, pst)
                    qts.append(qt_t)

                # ---- V projection (s-major) for this block's 4 s-tiles ----
                for j in range(4):
                    sidx = 4 * qb + j
                    psv = ps.tile([128, 512], F32, tag="mm512",
                                  name=f"pv{qb}_{j}")
                    for e in range(8):
                        nc.tensor.matmul(
                            psv,
                            xh[e // 4][:, e % 4, j * 128:(j + 1) * 128],
                            wv_sb[:, e, :],
                            start=(e == 0), stop=(e == 7),
                        )
                    vt = vp[sidx].rearrange("p (h c) -> p h c", c=DK + 1)
                    nc.vector.tensor_copy(
                        vt[:, :, 0:DK],
                        psv.rearrange("p (h d) -> p h d", d=DK),
                    )
                    nc.vector.tensor_copy(
                        vt[:, :, DK], ones_c.broadcast_to([128, 8])
                    )

                # ---- attention for this q-block ----
                ots = [xq.tile([128, 512], F32R, tag="ots", bufs=8,
                               name=f"ots{qb}_{t}") for t in range(4)]
                n_merge = 2 * qb + 2
                for pair in range(4):
                    hA, hB = 2 * pair, 2 * pair + 1
                    otA = ps.tile([DK + 1, 512], F32, tag="ot2",
                                  name=f"otA{qb}_{pair}")
                    otB = ps.tile([DK + 1, 512], F32, tag="ot2",
                                  name=f"otB{qb}_{pair}")
                    for m in range(n_merge):
                        stA = ps.tile([128, 1024], F32, tag="st",
                                      name=f"stA{qb}_{pair}_{m}")
                        stB = ps.tile([128, 1024], F32, tag="st",
                                      name=f"stB{qb}_{pair}_{m}")
                        for j in (0, 1):
                            ki = 2 * m + j
                            ksl = slice(ki * 128, (ki + 1) * 128)
                            nc.tensor.matmul(
                                stA[:, j * 512:(j + 1) * 512],
                                kt[pair][0:DK, ksl], qts[pair][0:DK, :],
                                start=True, stop=True, tile_position=(0, 0),
                            )
                            nc.tensor.matmul(
                                stB[:, j * 512:(j + 1) * 512],
                                kt[pair][DK:128, ksl], qts[pair][DK:128, :],
                                start=True, stop=True, tile_position=(64, 0),
                            )
                        ptA = wkp.tile([128, 1024], F32R, tag="pt",
                                       name=f"ptA{qb}_{pair}_{m}")
                        ptB = wkp.tile([128, 1024], F32R, tag="pt",
                                       name=f"ptB{qb}_{pair}_{m}")
                        nc.scalar.activation(ptA, stA, EXP, scale=SCALE)
                        nc.scalar.activation(ptB, stB, EXP, scale=SCALE)
                        if m >= n_merge - 2:
                            mi = m - (n_merge - 2)
                            nc.vector.tensor_mul(ptA, ptA, mask_sb[:, mi, :])
                            nc.vector.tensor_mul(ptB, ptB, mask_sb[:, mi, :])
                        for j in (0, 1):
                            ki = 2 * m + j
                            first = (m == 0 and j == 0)
                            last = (m == n_merge - 1 and j == 1)
                            nc.tensor.matmul(
                                otA, vp[ki][:, hA * 65:hA * 65 + 65],
                                ptA[:, j * 512:(j + 1) * 512],
                                start=first, stop=last,
                            )
                            nc.tensor.matmul(
                                otB, vp[ki][:, hB * 65:hB * 65 + 65],
                                ptB[:, j * 512:(j + 1) * 512],
                                start=first, stop=last,
                            )
                    for hl, ot_ps in ((0, otA), (1, otB)):
                        rb = wkp.tile([64, 512], F32, tag="rb",
                                      name=f"rb{qb}_{pair}_{hl}")
                        nc.vector.reciprocal(rb[0:1, :], ot_ps[DK:DK + 1, :])
                        nc.gpsimd.partition_broadcast(rb, rb[0:1, :])
                        nc.vector.tensor_mul(
                            ots[pair][hl * DK:(hl + 1) * DK, :],
                            ot_ps[0:DK, :], rb,
                        )

                # ---- output projection for this q-block's 4 s-tiles ----
                for j in range(4):
                    ps0 = ps.tile([128, 512], F32, tag="mm512",
                                  name=f"po0_{qb}_{j}")
                    ps1 = ps.tile([128, 512], F32, tag="mm512",
                                  name=f"po1_{qb}_{j}")
                    for di in range(4):
                        lhs = ots[di][:, j * 128:(j + 1) * 128]
                        nc.tensor.matmul(ps0, lhs, wo_sb[:, di, 0:512],
                                         start=(di == 0), stop=(di == 3))
                        nc.tensor.matmul(ps1, lhs, wo_sb[:, di, 512:1024],
                                         start=(di == 0), stop=(di == 3))
                    ostg = wkp.tile([128, 1024], F32, tag="ostg",
                                    name=f"ostg{qb}_{j}")
                    nc.vector.tensor_copy(ostg[:, 0:512], ps0)
                    nc.vector.tensor_copy(ostg[:, 512:1024], ps1)
                    sidx = 4 * qb + j
                    nc.sync.dma_start(
                        out=out_p[sidx * 128:(sidx + 1) * 128, :], in_=ostg
                    )

    nc.compile()
    _cache["nc"] = nc
    return nc


def _build_masks():
    # masks[i][kr, j*512 + qc] = 1 iff qc >= (2*i + j)*128 + kr
    m = np.zeros((2, 128, 1024), dtype=np.float32)
    kr = np.arange(128)[:, None]
    qc = np.arange(512)[None, :]
    for i in range(2):
        for j in range(2):
            m[i, :, j * 512:(j + 1) * 512] = (qc >= (2 * i + j) * 128 + kr)
    return m


def kernel(x, w_q, w_k, w_v, w_o, b_o):
    x = np.asarray(x, dtype=np.float32)
    w_q = np.asarray(w_q, dtype=np.float32)
    w_k = np.asarray(w_k, dtype=np.float32)
    w_v = np.asarray(w_v, dtype=np.float32)
    w_o = np.asarray(w_o, dtype=np.float32)
    b_o = np.asarray(b_o, dtype=np.float32)

    nc = _build()
    masks = _build_masks()

    in_maps = []
    for core in range(N_CORES):
        b, g = core // 2, core % 2
        sl = slice(g * GD, (g + 1) * GD)
        in_maps.append({
            "xT": np.ascontiguousarray(x[b].T),
            "wq_t": np.ascontiguousarray(w_q[sl, :].T),
            "wk_t": np.ascontiguousarray(w_k[sl, :].T),
            "wv_t": np.ascontiguousarray(w_v[sl, :].T),
            "wo_t": np.ascontiguousarray(w_o[:, sl].T),
            "masks": masks,
        })

    res = run_bass_kernel_spmd(nc, in_maps, core_ids=list(range(N_CORES)),
                               trace=False)

    out = np.empty((B, S, D), dtype=np.float32)
    for b in range(B):
        out[b] = res.results[2 * b]["out_p"] + res.results[2 * b + 1]["out_p"]
    out += b_o[None, None, :]
    return out


# revision 6
# speedup vs baseline: 1.0097x; 1.0097x over previous
"""Multi-head self-attention (B=4, S=2048, D=1024, H=16, causal) on 8 TRN2
NeuronCores.

Sharding: batch x head-group. Core c handles batch b = c//2 and head-group
g = c%2 (8 heads = 512 of the 1024 q/k/v dims). Each core computes a partial
output [S, D] (its head-group's contribution through w_o); the host sums the
two partials per batch and adds b_o.

Per-core kernel (all matmuls fp32r = TF32-like, fp32 accumulate):
  for each q-block (512 queries):
    - project Q (this block) and K/V (this block's keys) from x
    - flash-style attention over k-tiles <= q-block end:
        S^T[k,q] = Kt.T @ Qt  (two heads packed in the PE array via
        tile_position row groups), merged exp on ACT with fused 1/sqrt(dk)
        scale (no max subtraction: |scores| <~ 6 so exp is safe), causal mask
        via multiply on the diagonal tiles, O'^T += V'.T @ P^T where V' has a
        ones column appended so the softmax denominator accumulates for free
    - normalize O^T by the denominator (reciprocal + partition broadcast)
    - output projection of this q-block, DMA partial out
"""

import numpy as np

import concourse.bass as bass
import concourse.mybir as mybir
from concourse import bacc
from concourse.tile import TileContext
from concourse.bass_utils import run_bass_kernel_spmd

B, S, D, H = 4, 2048, 1024, 16
DK = D // H          # 64
N_CORES = 8
GD = D // 2          # 512 dims per head-group
SCALE = 1.0 / float(np.sqrt(DK))

F32 = mybir.dt.float32
F32R = mybir.dt.float32r
EXP = mybir.ActivationFunctionType.Exp

_cache = {}


def _build():
    if "nc" in _cache:
        return _cache["nc"]

    nc = bacc.Bacc("TRN2", target_bir_lowering=False, debug=False,
                   num_devices=N_CORES)

    xT = nc.dram_tensor("xT", (D, S), F32R, kind="ExternalInput")
    wq_t = nc.dram_tensor("wq_t", (D, GD), F32R, kind="ExternalInput")
    wk_t = nc.dram_tensor("wk_t", (D, GD), F32R, kind="ExternalInput")
    wv_t = nc.dram_tensor("wv_t", (D, GD), F32R, kind="ExternalInput")
    wo_t = nc.dram_tensor("wo_t", (GD, D), F32R, kind="ExternalInput")
    masks = nc.dram_tensor("masks", (2, 128, 1024), F32, kind="ExternalInput")
    out_p = nc.dram_tensor("out_p", (S, D), F32, kind="ExternalOutput")

    xT_r = xT.rearrange("(t p) s -> p t s", p=128)        # [128, 8, 2048]
    wq_r = wq_t.rearrange("(t p) d -> p t d", p=128)      # [128, 8, 512]
    wk_r = wk_t.rearrange("(t p) d -> p t d", p=128)
    wv_r = wv_t.rearrange("(t p) d -> p t d", p=128)
    wo_r = wo_t.rearrange("(t p) d -> p t d", p=128)      # [128, 4, 1024]

    with TileContext(nc) as tc:
        with (
            tc.tile_pool(name="pers", bufs=1) as pers,
            tc.tile_pool(name="wp", bufs=1) as wp,
            tc.tile_pool(name="xq", bufs=2) as xq,
            tc.tile_pool(name="wkp", bufs=2) as wkp,
            tc.tile_pool(name="ps", bufs=2, space="PSUM") as ps,
        ):
            # persistent K^T (d-major) and V' (s-major, 65 cols/head)
            kt = [pers.tile([128, S], F32R, name=f"kt{t}") for t in range(4)]
            vp = [pers.tile([128, 8 * (DK + 1)], F32R, name=f"vp{i}")
                  for i in range(16)]

            wq_sb = wp.tile([128, 8, GD], F32R)
            wk_sb = wp.tile([128, 8, GD], F32R)
            wv_sb = wp.tile([128, 8, GD], F32R)
            wo_sb = wp.tile([128, 4, D], F32R)
            mask_sb = wp.tile([128, 2, 1024], F32)
            ones_c = wp.tile([128, 1], F32)
            nc.sync.dma_start(out=wq_sb, in_=wq_r)
            nc.sync.dma_start(out=wk_sb, in_=wk_r)
            nc.sync.dma_start(out=wv_sb, in_=wv_r)
            nc.sync.dma_start(out=wo_sb, in_=wo_r)
            nc.sync.dma_start(out=mask_sb, in_=masks.rearrange("i p q -> p i q"))
            nc.vector.memset(ones_c, 1.0)

            for qb in range(4):
                qs = slice(qb * 512, (qb + 1) * 512)

                # ---- x chunk for this q/s block, in two halves ----
                xh = []
                for h in range(2):
                    xt = xq.tile([128, 4, 512], F32R, tag="xh", name=f"xh{qb}_{h}")
                    nc.sync.dma_start(out=xt, in_=xT_r[:, 4 * h:4 * h + 4, qs])
                    xh.append(xt)

                # ---- Q / K projections for this block ----
                qts = []
                for t in range(4):
                    qt_t = xq.tile([128, 512], F32R, tag="qts", bufs=8,
                                   name=f"qts{qb}_{t}")
                    for dst, wsb in ((qt_t, wq_sb), (None, wk_sb)):
                        pst = ps.tile([128, 512], F32, tag="mm512",
                                      name=f"pp{qb}_{t}")
                        for e in range(8):
                            nc.tensor.matmul(
                                pst,
                                wsb[:, e, t * 128:(t + 1) * 128],
                                xh[e // 4][:, e % 4, :],
                                start=(e == 0), stop=(e == 7),
                            )
                        if dst is None:
                            nc.vector.tensor_copy(kt[t][:, qs], pst)
                        else:
                            nc.vector.tensor_copy(dst, pst)
                    qts.append(qt_t)

                # ---- V projection (s-major) for this block's 4 s-tiles ----
                for j in range(4):
                    sidx = 4 * qb + j
                    psv = ps.tile([128, 512], F32, tag="mm512",
                                  name=f"pv{qb}_{j}")
                    for e in range(8):
                        nc.tensor.matmul(
                            psv,
                            xh[e // 4][:, e % 4, j * 128:(j + 1) * 128],
                            wv_sb[:, e, :],
                            start=(e == 0), stop=(e == 7),
                        )
                    vt = vp[sidx].rearrange("p (h c) -> p h c", c=DK + 1)
                    nc.vector.tensor_copy(
                        vt[:, :, 0:DK],
                        psv.rearrange("p (h d) -> p h d", d=DK),
                    )
                    nc.vector.tensor_copy(
                        vt[:, :, DK], ones_c.broadcast_to([128, 8])
                    )

                # ---- attention for this q-block ----
                ots = [xq.tile([128, 512], F32R, tag="ots", bufs=8,
                               name=f"ots{qb}_{t}") for t in range(4)]
                n_merge = 2 * qb + 2
                for pair in range(4):
                    hA, hB = 2 * pair, 2 * pair + 1
                    otA = ps.tile([DK + 1, 512], F32, tag="ot2",
                                  name=f"otA{qb}_{pair}")
                    otB = ps.tile([DK + 1, 512], F32, tag="ot2",
                                  name=f"otB{qb}_{pair}")
                    for m in range(n_merge):
                        stA = ps.tile([128, 1024], F32, tag="st",
                                      name=f"stA{qb}_{pair}_{m}")
                        stB = ps.tile([128, 1024], F32, tag="st",
                                      name=f"stB{qb}_{pair}_{m}")
                        for j in (0, 1):
                            ki = 2 * m + j
                            ksl = slice(ki * 128, (ki + 1) * 128)
                            nc.tensor.matmul(
                                stA[:, j * 512:(j + 1) * 512],
                                kt[pair][0:DK, ksl], qts[pair][0:DK, :],
                                start=True, stop=True, tile_position=(0, 0),
                            )
                            nc.tensor.matmul(
                                stB[:, j * 512:(j + 1) * 512],
                                kt[pair][DK:128, ksl], qts[pair][DK:128, :],
                                start=True, stop=True, tile_position=(64, 0),
                            )
                        ptA = wkp.tile([128, 1024], F32R, tag="pt",
                                       name=f"ptA{qb}_{pair}_{m}")
                        ptB = wkp.tile([128, 1024], F32R, tag="pt",
                                       name=f"ptB{qb}_{pair}_{m}")
                        nc.scalar.activation(ptA, stA, EXP, scale=SCALE)
                        nc.scalar.activation(ptB, stB, EXP, scale=SCALE)
                        if m >= n_merge - 2:
                            mi = m - (n_merge - 2)
                            nc.vector.tensor_mul(ptA, ptA, mask_sb[:, mi, :])
                            nc.vector.tensor_mul(ptB, ptB, mask_sb[:, mi, :])
                        for j in (0, 1):
                            ki = 2 * m + j
                            first = (m == 0 and j == 0)
                            last = (m == n_merge - 1 and j == 1)
                            nc.tensor.matmul(
                                otA, vp[ki][:, hA * 65:hA * 65 + 65],
                                ptA[:, j * 512:(j + 1) * 512],
                                start=first, stop=last,
                            )
                            nc.tensor.matmul(
                                otB, vp[ki][:, hB * 65:hB * 65 + 65],
                                ptB[:, j * 512:(j + 1) * 512],
                                start=first, stop=last,
                            )
                    for hl, ot_ps in ((0, otA), (1, otB)):
                        rb = wkp.tile([64, 512], F32, tag="rb",
                                      name=f"rb{qb}_{pair}_{hl}")
                        rc = wkp.tile([1, 512], F32, tag="rc", bufs=1,
                                      name=f"rc{qb}_{pair}_{hl}")
                        nc.vector.tensor_copy(rc, ot_ps[DK:DK + 1, :])
                        nc.vector.reciprocal_approx_fast(rb[0:1, :], rc)
                        nc.gpsimd.partition_broadcast(rb, rb[0:1, :])
                        nc.vector.tensor_mul(
                            ots[pair][hl * DK:(hl + 1) * DK, :],
                            ot_ps[0:DK, :], rb,
                        )

                # ---- output projection for this q-block's 4 s-tiles ----
                for j in range(4):
                    ps0 = ps.tile([128, 512], F32, tag="mm512",
                                  name=f"po0_{qb}_{j}")
                    ps1 = ps.tile([128, 512], F32, tag="mm512",
                                  name=f"po1_{qb}_{j}")
                    for di in range(4):
                        lhs = ots[di][:, j * 128:(j + 1) * 128]
                        nc.tensor.matmul(ps0, lhs, wo_sb[:, di, 0:512],
                                         start=(di == 0), stop=(di == 3))
                        nc.tensor.matmul(ps1, lhs, wo_sb[:, di, 512:1024],
                                         start=(di == 0), stop=(di == 3))
                    ostg = wkp.tile([128, 1024], F32, tag="ostg",
                                    name=f"ostg{qb}_{j}")
                    nc.vector.tensor_copy(ostg[:, 0:512], ps0)
                    nc.vector.tensor_copy(ostg[:, 512:1024], ps1)
                    sidx = 4 * qb + j
                    nc.sync.dma_start(
                        out=out_p[sidx * 128:(sidx + 1) * 128, :], in_=ostg
                    )

    nc.compile()
    _cache["nc"] = nc
    return nc


def _build_masks():
    # masks[i][kr, j*512 + qc] = 1 iff qc >= (2*i + j)*128 + kr
    m = np.zeros((2, 128, 1024), dtype=np.float32)
    kr = np.arange(128)[:, None]
    qc = np.arange(512)[None, :]
    for i in range(2):
        for j in range(2):
            m[i, :, j * 512:(j + 1) * 512] = (qc >= (2 * i + j) * 128 + kr)
    return m


def kernel(x, w_q, w_k, w_v, w_o, b_o):
    x = np.asarray(x, dtype=np.float32)
    w_q = np.asarray(w_q, dtype=np.float32)
    w_k = np.asarray(w_k, dtype=np.float32)
    w_v = np.asarray(w_v, dtype=np.float32)
    w_o = np.asarray(w_o, dtype=np.float32)
    b_o = np.asarray(b_o, dtype=np.float32)

    nc = _build()
    masks = _build_masks()

    in_maps = []
    for core in range(N_CORES):
        b, g = core // 2, core % 2
        sl = slice(g * GD, (g + 1) * GD)
        in_maps.append({
            "xT": np.ascontiguousarray(x[b].T),
            "wq_t": np.ascontiguousarray(w_q[sl, :].T),
            "wk_t": np.ascontiguousarray(w_k[sl, :].T),
            "wv_t": np.ascontiguousarray(w_v[sl, :].T),
            "wo_t": np.ascontiguousarray(w_o[:, sl].T),
            "masks": masks,
        })

    res = run_bass_kernel_spmd(nc, in_maps, core_ids=list(range(N_CORES)),
                               trace=False)

    out = np.empty((B, S, D), dtype=np.float32)
    for b in range(B):
        out[b] = res.results[2 * b]["out_p"] + res.results[2 * b + 1]["out_p"]
    out += b_o[None, None, :]
    return out
